# revision 1
# baseline (speedup 1.0000x reference)
"""Bass/Trainium2 kernel for nn_AttentionOutRNNUnit.

  Wh[b]     = W @ hidden[b]                      (E x H @ H -> E)
  scores[b] = enc[b] @ Wh[b]                     (L x E @ E -> L)
  attn[b]   = softmax(scores[b] + bias)          (softmax over L; uniform bias
                                                  shift cancels exactly)
  out[b]    = enc[b]^T @ attn[b]                 (E)

Data-parallel over batch: 8 batches per NeuronCore, 8 cores. Per batch,
enc[b] ([4096, 256] f32 = 4 MB) streams once into SBUF in natural layout
[128 l-partitions, 32 l-tiles, 256 e]. Raw-bass engine pipeline (the
vendored walrus rejects instructions with >1 inline sync wait and all
custom-DVE/ISA ops, so Tile and the fused reduce ops are unusable;
explicit standalone wait_ge instructions are used instead):

  SP:     enc[b] DMA (4 MB each, double-buffered), wconst DMA, out DMA
  PE:     Wh[b] broadcast (hidrep^T @ W^T -> all 128 partitions),
          softmax denominator (sumpart^T @ ones), context accumulation
          (attn column [128,1] stationary x enc tile [128,256] moving,
          PSUM-accumulated over 32 l-tiles)
  DVE:    P = enc * Wh[b] (one [128, 32, 256] tensor_tensor), then the
          scores reduction for l-tiles [0, KDVE) via fused
          tensor_scalar(x*1+0, accum_out) at the fp32 2x perf mode,
          reciprocal of the denominator
  ACT:    scores reduction for l-tiles [KDVE, 32) via
          activation(Copy, accum_out), whb PSUM->SBUF copy,
          attn = exp(scores) with accum_out partial sums,
          final context * (1/denom)

GpSimd is intentionally unused: its tensor ops share an SBUF port with
DVE and measurably slow the DVE stream.

`repeats` > 1 re-runs the whole batch sweep back-to-back inside one NEFF
(semaphore thresholds keep counting; the pipeline never drains), used for
wall-clock timing of the steady state.
"""

import numpy as np

B, L, E, H = 64, 4096, 256, 256
NCORES = 8
BPC = B // NCORES  # batches per core
P = 128            # partitions
LT = L // P        # l-tiles per batch
KT = H // P        # contraction tiles for the Wh matmul

WT_COLS = KT * E
HID_COLS = BPC * KT * P
ONES_COL = WT_COLS + HID_COLS
ZERO_COL = ONES_COL + 1
WCONST_COLS = ZERO_COL + 1

KDVE = 13  # scores columns reduced on DVE; the rest on ACT

TRACE = False
LAST_RESULT = None


def build_bass(repeats=1, ctx_f32r=False, kdve=None):
    import concourse.bass as bass
    import concourse.mybir as mybir

    f32 = mybir.dt.float32
    Alu = mybir.AluOpType
    Act = mybir.ActivationFunctionType
    kd = KDVE if kdve is None else kdve

    nc = bass.Bass()

    f32r = mybir.dt.float32r
    enc_dt = f32r if ctx_f32r else f32

    # enc is host-pre-arranged to partition-major [p, t, e] per batch so the
    # DMA streams one contiguous 32 KB run per partition (HBM-friendly).
    enc_d = nc.dram_tensor("enc", [BPC, P, LT, E], f32, kind="ExternalInput")
    wconst_d = nc.dram_tensor("wconst", [P, WCONST_COLS], f32, kind="ExternalInput")
    out_d = nc.dram_tensor("out", [1, BPC * E], f32, kind="ExternalOutput")

    enc_sb = [
        nc.alloc_sbuf_tensor(f"enc_sb{s}", [P, LT, E], enc_dt) for s in range(2)
    ]
    pfull = [nc.alloc_sbuf_tensor(f"pfull{s}", [P, LT, E], f32) for s in range(2)]
    wconst_sb = nc.alloc_sbuf_tensor("wconst_sb", [P, WCONST_COLS], f32)
    whb = [nc.alloc_sbuf_tensor(f"whb{s}", [P, 1, E], f32) for s in range(2)]
    scores = [nc.alloc_sbuf_tensor(f"scores{s}", [P, LT], f32) for s in range(2)]
    attn = [nc.alloc_sbuf_tensor(f"attn{s}", [P, LT], enc_dt) for s in range(2)]
    sumpart = [nc.alloc_sbuf_tensor(f"sumpart{s}", [P, 1], f32) for s in range(2)]
    recip = [nc.alloc_sbuf_tensor(f"recip{s}", [1, 1], f32) for s in range(2)]
    out_sb = nc.alloc_sbuf_tensor("out_sb", [1, BPC * E], f32)

    ps_rep = [nc.alloc_psum_tensor(f"ps_rep{s}", [P, E], f32) for s in range(2)]
    ps_den = [nc.alloc_psum_tensor(f"ps_den{s}", [1, 1], f32) for s in range(2)]
    ps_ctx = [nc.alloc_psum_tensor(f"ps_ctx{s}", [1, E], f32) for s in range(2)]

    ones_col = wconst_sb[:, ONES_COL : ONES_COL + 1]
    zero_col = wconst_sb[:, ZERO_COL : ZERO_COL + 1]

    def wt_view(kt):
        return wconst_sb[:, kt * E : (kt + 1) * E]

    def hid_view(b, kt):
        off = WT_COLS + ((b % BPC) * KT + kt) * P
        return wconst_sb[:, off : off + P]

    NB = repeats * BPC  # total virtual batches

    s_const = nc.alloc_semaphore("s_const")
    s_encb = [nc.alloc_semaphore(f"s_enc{b}") for b in range(BPC)]
    s_rep = nc.alloc_semaphore("s_rep")
    s_whb = nc.alloc_semaphore("s_whb")
    s_p = nc.alloc_semaphore("s_p")
    s_sc = nc.alloc_semaphore("s_sc")
    s_sca = nc.alloc_semaphore("s_sca")
    s_attn = nc.alloc_semaphore("s_attn")
    s_den = nc.alloc_semaphore("s_den")
    s_rec = nc.alloc_semaphore("s_rec")
    s_ctx = nc.alloc_semaphore("s_ctx")
    s_out = nc.alloc_semaphore("s_out")
    s_fin = nc.alloc_semaphore("s_fin")

    with nc.Block() as block:

        @block.sync
        def _(sync: bass.BassEngine):
            sync.dma_start(out=wconst_sb[:, :], in_=wconst_d[:, :]).then_inc(
                s_const, 16
            )
            for vb in range(NB):
                if vb >= 2:
                    sync.wait_ge(s_p, vb - 1)
                    sync.wait_ge(s_ctx, vb - 1)
                if vb >= BPC:
                    # same per-batch sem reused across repeats; ensure the
                    # previous epoch's increments fully landed before adding
                    sync.wait_ge(s_encb[vb % BPC], 16 * (vb // BPC))
                enc_src = enc_d[vb % BPC][:, :, :]
                if ctx_f32r:
                    enc_src = enc_src.bitcast(f32r)
                sync.dma_start(
                    out=enc_sb[vb % 2][:, :, :],
                    in_=enc_src,
                ).then_inc(s_encb[vb % BPC], 16)
            sync.wait_ge(s_out, NB)
            sync.dma_start(out=out_d[:, :], in_=out_sb[:, :]).then_inc(s_fin, 16)
            sync.wait_ge(s_fin, 16)

        @block.tensor
        def _(pe: bass.BassEngine):
            t_ = nc.tensor

            def bcast(vb):
                for kt in range(KT):
                    mm = t_.matmul(
                        out=ps_rep[vb % 2][:, :],
                        lhsT=hid_view(vb, kt),
                        rhs=wt_view(kt),
                        start=(kt == 0),
                        stop=(kt == KT - 1),
                    )
                mm.then_inc(s_rep, 1)

            pe.wait_ge(s_const, 16)
            bcast(0)
            if NB > 1:
                bcast(1)
            for vb in range(NB):
                s = vb % 2
                if vb + 2 < NB:
                    pe.wait_ge(s_whb, vb + 1)
                    bcast(vb + 2)
                pe.wait_ge(s_attn, vb + 1)
                if vb >= 2:
                    pe.wait_ge(s_rec, vb - 1)
                t_.matmul(
                    out=ps_den[s][:, :],
                    lhsT=sumpart[s][:, :],
                    rhs=ones_col,
                    start=True,
                    stop=True,
                ).then_inc(s_den, 1)
                if vb >= 2:
                    pe.wait_ge(s_out, vb - 1)
                for t in range(LT):
                    mm = t_.matmul(
                        out=ps_ctx[s][:, :],
                        lhsT=attn[s][:, t : t + 1],
                        rhs=enc_sb[s][:, t, :],
                        start=(t == 0),
                        stop=(t == LT - 1),
                    )
                mm.then_inc(s_ctx, 1)

        @block.vector
        def _(v: bass.BassEngine):
            vec = nc.vector
            for vb in range(NB):
                s = vb % 2
                v.wait_ge(s_encb[vb % BPC], 16 * (vb // BPC + 1))
                v.wait_ge(s_whb, vb + 1)
                if vb >= 2:
                    # scores slot free AND ACT's pfull reads of vb-2 done
                    v.wait_ge(s_attn, vb - 1)
                enc_in = enc_sb[s][:, :, :]
                if ctx_f32r:
                    enc_in = enc_in.bitcast(f32)
                vec.tensor_tensor(
                    out=pfull[s][:, :, :],
                    in0=enc_in,
                    in1=whb[s][:, :, :].broadcast_to((P, LT, E)),
                    op=Alu.mult,
                ).then_inc(s_p, 1)
                v.wait_ge(s_p, vb + 1)
                for t in range(kd):
                    mm = vec.tensor_scalar(
                        out=pfull[s][:, t, :],
                        in0=pfull[s][:, t, :],
                        scalar1=1.0,
                        scalar2=0.0,
                        op0=Alu.mult,
                        op1=Alu.add,
                        accum_out=scores[s][:, t : t + 1],
                    )
                mm.then_inc(s_sc, 1)
                if vb >= 1:
                    j = vb - 1
                    v.wait_ge(s_den, j + 1)
                    if j >= 2:
                        v.wait_ge(s_out, j - 1)
                    vec.reciprocal(recip[j % 2][:, :], ps_den[j % 2][:, :]).then_inc(
                        s_rec, 1
                    )
            j = NB - 1
            v.wait_ge(s_den, j + 1)
            vec.reciprocal(recip[j % 2][:, :], ps_den[j % 2][:, :]).then_inc(s_rec, 1)

        @block.scalar
        def _(act: bass.BassEngine):
            sc = nc.scalar

            def whb_copy(vb):
                sc.activation(
                    out=whb[vb % 2][:, 0, :],
                    in_=ps_rep[vb % 2][:, :],
                    func=Act.Copy,
                ).then_inc(s_whb, 1)

            act.wait_ge(s_rep, 1)
            whb_copy(0)
            if NB > 1:
                act.wait_ge(s_rep, 2)
                whb_copy(1)
            for vb in range(NB):
                s = vb % 2
                if vb + 2 < NB:
                    act.wait_ge(s_rep, vb + 3)
                    act.wait_ge(s_p, vb + 1)
                    whb_copy(vb + 2)
                else:
                    act.wait_ge(s_p, vb + 1)
                # ACT's share of the scores reduction (in-place copy + accum)
                if kd < LT:
                    for t in range(kd, LT):
                        ai = sc.activation(
                            out=pfull[s][:, t, :],
                            in_=pfull[s][:, t, :],
                            func=Act.Copy,
                            accum_out=scores[s][:, t : t + 1],
                        )
                    ai.then_inc(s_sca, 1)
                else:
                    act.sem_inc(s_sca, 1)
                act.wait_ge(s_sca, vb + 1)
                act.wait_ge(s_sc, vb + 1)
                if vb >= 2:
                    act.wait_ge(s_ctx, vb - 1)
                    act.wait_ge(s_den, vb - 1)
                sc.activation(
                    out=attn[s][:, :],
                    in_=scores[s][:, :],
                    func=Act.Exp,
                    bias=zero_col,
                    scale=1.0,
                    accum_out=sumpart[s][:, :],
                ).then_inc(s_attn, 1)
                act.wait_ge(s_ctx, vb + 1)
                act.wait_ge(s_rec, vb + 1)
                sc.activation(
                    out=out_sb[:, (vb % BPC) * E : (vb % BPC + 1) * E],
                    in_=ps_ctx[s][:, :],
                    func=Act.Copy,
                    scale=recip[s][:, :],
                ).then_inc(s_out, 1)

    return nc


def make_wconst(hidden_shard, W):
    """Pack W^T tiles, replicated hidden, ones and zeros into one [P, C] f32."""
    wt_part = W.T.reshape(KT, P, E).transpose(1, 0, 2).reshape(P, WT_COLS)
    hid_part = np.broadcast_to(
        hidden_shard.T.reshape(KT, P, BPC).transpose(1, 2, 0)[:, :, :, None],
        (P, BPC, KT, P),
    ).reshape(P, HID_COLS)
    ones = np.ones((P, 1), dtype=np.float32)
    zeros = np.zeros((P, 1), dtype=np.float32)
    return np.ascontiguousarray(
        np.concatenate([wt_part, hid_part, ones, zeros], axis=1, dtype=np.float32)
    )


def make_in_maps(hidden, encoderhidden, W):
    in_maps = []
    for i in range(NCORES):
        sl = slice(i * BPC, (i + 1) * BPC)
        enc_pt = np.ascontiguousarray(
            encoderhidden[sl].reshape(BPC, LT, P, E).transpose(0, 2, 1, 3)
        )
        in_maps.append(
            {
                "enc": enc_pt,
                "wconst": make_wconst(hidden[sl], W),
            }
        )
    return in_maps


def kernel(hidden, encoderhidden, W, b):
    """Full (unsharded) inputs in, full output out.

    The additive bias b enters scores uniformly, so softmax cancels it
    exactly; it is not shipped to the device.
    """
    global LAST_RESULT
    from concourse.bass_utils import run_bass_kernel_spmd

    hidden = np.asarray(hidden, dtype=np.float32)
    encoderhidden = np.asarray(encoderhidden, dtype=np.float32)
    W = np.asarray(W, dtype=np.float32)

    nc = build_bass()
    in_maps = make_in_maps(hidden, encoderhidden, W)

    res = run_bass_kernel_spmd(nc, in_maps, list(range(NCORES)), trace=TRACE)
    LAST_RESULT = res

    out = np.concatenate(
        [res.results[i]["out"].reshape(BPC, E) for i in range(NCORES)], axis=0
    )
    return out

